# revision 12
# baseline (speedup 1.0000x reference)
"""Trainium2 Bass kernel for nn_ChannelWiseQuantumKernel (v2).

Method (per-pixel phase precompute + compact z/w state recurrence):

The per-position data RY gates are diagonal phase gates in the (SH)-transformed
basis: the 16 phase angles per patch-position are SHIFTED READS of a per-PIXEL
field alpha_j(img, h, w) = sum_ch +-theta_ch/2 (a 4->16 signed channel mix).
So all trig is computed once per pixel instead of once per patch-position
(9x less work): PhiC = -cos(alpha)/4, PhiS = -sin(alpha)/4 stored as fp16
pixel arrays [128, 2, 64, 64] (partitions = kernel(2) x img(2) x state(32)).

State: compact 32 real values per (kernel, img): y = [yR(16); yI(16)], all
four (k, img) groups packed in 128 partitions. Per position p:
    z = PhiC_view(p) * y ;  w = PhiS_view(p) * y      (one fused DVE TT, 2x
                                                       fp16 mode, strided
                                                       shifted-window views)
    y' = M1_p @ z + M2_p @ w                          (two accumulated fp16
                                                       128x128 matmuls)
with M1 = -4*[[Gr,-Gi],[Gi,Gr]], M2 = -4*[[-Gi,-Gr],[Gr,-Gi]] per kernel.
A ScalarE copy downconverts y (fp32 PSUM) to fp16 SBUF each position so the
DVE multiply runs in the packed 2x mode. Measurement: Square + one
128->16 signed-sum matmul; 9 positions -> <Z_q> for 2 kernels x 2 images.
Patches stream in 8 column-chunks of 8 output rows (496 cols) per core.
"""

import sys

sys.path.insert(0, "/opt/trn_rl_repo")

import numpy as np

import concourse.bacc as bacc
import concourse.bass as bass
import concourse.tile as tile
from concourse import mybir
from concourse.bass_utils import run_bass_kernel_spmd

# ---------------- problem constants ----------------
IN_CH = 4
KSZ = 3
NPOS = 9
DIM = 16
B = 16
HW = 64
OH = HW - KSZ + 1  # 62
P = OH * OH  # 3844 patches per image
N_CORES = 8
NPIX = HW * HW  # 4096
PIXCH = 512  # pixels per pixel-chunk (8 rows of 64)
ROWS_PER_CHUNK = 8  # output rows per patch-chunk
N_CHAINS = 7  # patch-chunk chains in flight (1 PSUM bank each)
DVE_CV_POS = (3, 6)  # positions with fused direct-PSUM multiply on VectorE
POOL_TT_POS = (2, 5, 7)  # positions whose phase-multiply runs on GpSimd

# wm (fp16 weight matrix) column layout
A_ALPHA = 0
A_P0C = 128
A_P0S = 256
A_M = 384  # 16 x 128 cols: (p-1)*256 + {0 (M1), 128 (M2)}
A_Z = A_M + 16 * 128  # 2432
WCOLS = A_Z + 16  # 2448

# ---------------- host-side constant math (weights-only, O(1)) ----------------
_H = np.array([[1, 1], [1, -1]], dtype=np.complex128) / np.sqrt(2)
_S = np.array([[1, 0], [0, 1j]], dtype=np.complex128)
_A1 = _S @ _H


def _kron_n(mats):
    out = np.array([[1.0 + 0j]])
    for m in mats:
        out = np.kron(out, m)
    return out


_AA = _kron_n([_A1] * IN_CH)
_U1 = _A1.conj().T @ (np.array([1.0, 1.0]) / np.sqrt(2))
_YINIT = _kron_n([_U1.reshape(2, 1)] * IN_CH).reshape(DIM)

_SGN = np.array(
    [[2 * ((i >> (3 - ch)) & 1) - 1 for i in range(DIM)] for ch in range(IN_CH)],
    dtype=np.float64,
)
_SIGMA = np.array(
    [[1 - 2 * ((i >> (3 - q)) & 1) for i in range(DIM)] for q in range(IN_CH)],
    dtype=np.float64,
)


def _rx(t):
    c, s = np.cos(t / 2), np.sin(t / 2)
    return np.array([[c, -1j * s], [-1j * s, c]])


def _ry(t):
    c, s = np.cos(t / 2), np.sin(t / 2)
    return np.array([[c, -s], [s, c]])


def _rz(t):
    e = np.exp(-0.5j * t)
    return np.array([[e, 0], [0, np.conj(e)]])


def _embed(U, q):
    mats = [np.eye(2, dtype=complex)] * IN_CH
    mats[q] = U
    return _kron_n(mats)


def _cx(cq, tq):
    M = np.zeros((DIM, DIM), dtype=complex)
    for i in range(DIM):
        bits = [(i >> (3 - q)) & 1 for q in range(4)]
        j = i
        if bits[cq] == 1:
            bits2 = bits.copy()
            bits2[tq] ^= 1
            j = sum(b << (3 - q) for q, b in enumerate(bits2))
        M[j, i] = 1
    return M


def _build_G(w_flat):
    w = np.float64(w_flat).reshape(NPOS, 1, IN_CH, 3)
    Gs = []
    for pos in range(NPOS):
        U = np.eye(DIM, dtype=complex)
        for q in range(IN_CH):
            R = _rz(w[pos, 0, q, 2]) @ _ry(w[pos, 0, q, 1]) @ _rx(w[pos, 0, q, 0])
            U = _embed(R, q) @ U
        for q in range(IN_CH - 1):
            U = _cx(q, q + 1) @ U
        U = _cx(IN_CH - 1, 0) @ U
        Gs.append(_AA.conj().T @ U @ _AA)
    G8f = _AA @ Gs[8]
    return Gs, G8f


def _build_wm(weights):
    """All matmul lhsT constants, fp16 [128, WCOLS].

    Partition groups g = k*2 + img occupy partitions [32g, 32g+32)."""
    wm = np.zeros((128, WCOLS), dtype=np.float64)
    # alpha lhsT: rows (img*4+ch) 0..8, cols (g, j32); emits alpha/3 directly
    for k in range(2):
        for img in range(2):
            g = k * 2 + img
            for ch in range(IN_CH):
                for j in range(32):
                    wm[img * 4 + ch, A_ALPHA + g * 32 + j] = _SGN[ch, j % 16] / 6.0
    for k in range(2):
        Gs, G8f = _build_G(weights[k])
        G0c = Gs[0] @ np.diag(_YINIT)
        Pm, Qm = G0c.real, G0c.imag
        Ac = np.vstack([Pm, Qm])  # 32x16: c16 -> [y1R; y1I]
        As = np.vstack([-Qm, Pm])
        for img in range(2):
            g = k * 2 + img
            r0 = g * 32
            # pos0: lhsT[r0+j, A_P0C + r0+o] = -4*Ac[o, j] (j<16)
            wm[r0 : r0 + 16, A_P0C + r0 : A_P0C + r0 + 32] = -4.0 * Ac.T
            wm[r0 : r0 + 16, A_P0S + r0 : A_P0S + r0 + 32] = -4.0 * As.T
        for p in range(1, 9):
            G = G8f if p == 8 else Gs[p]
            Gr, Gi = G.real, G.imag
            M1 = np.block([[Gr, -Gi], [Gi, Gr]])
            M2 = np.block([[-Gi, -Gr], [Gr, -Gi]])
            col = A_M + (p - 1) * 256
            for img in range(2):
                g = k * 2 + img
                r0 = g * 32
                wm[r0 : r0 + 32, col + r0 : col + r0 + 32] = -4.0 * M1.T
                wm[r0 : r0 + 32, col + 128 + r0 : col + 128 + r0 + 32] = -4.0 * M2.T
        # Z lhsT: out row r = k*8 + img*4 + q
        for img in range(2):
            g = k * 2 + img
            r0 = g * 32
            for q in range(IN_CH):
                for j in range(32):
                    wm[r0 + j, A_Z + k * 8 + img * 4 + q] = _SIGMA[q, j % 16]
    return wm.astype(np.float16)


# ---------------- custom fused DVE op: out = ((v^2 - 3/4) * v) * y ----------
_CUBE_OP = None


def _register_cube_mul():
    """Register the fused triple-angle multiply as a custom DVE op.

    One VectorE instruction computes ((v*v - c0) * v) * y, which applies the
    full range-reduced phase factor to the state in a single pass."""
    global _CUBE_OP
    if _CUBE_OP is not None:
        return _CUBE_OP
    import concourse.dve_ops as dve_ops

    for o in dve_ops.OPS:
        if o.name == "CUBE_MUL_ANT":
            _CUBE_OP = o
            return o
    from concourse.dve_ops import DveOp
    from concourse.dve_spec import C0, Spec, Src0, Src1, lower
    from concourse.dve_uop import DveOpSpec

    body = ((Src0 * Src0 - C0) * Src0) * Src1
    spec = Spec(
        body=body,
        reference=lambda in0, in1, c0, c1, c2: (
            ((in0.astype(np.float32) * in0 - c0) * in0) * in1
        ),
    )
    row = max(dve_ops._SUB_OPCODE_FOR_NAME.values()) + 1
    shas = {}
    for ver in ("v3", "v4"):
        uops = lower(spec, ver=ver)
        shas[ver] = DveOpSpec(
            name="CUBE_MUL_ANT", opcode=row, uops=uops, rd1_en=True
        ).sha(ver)
    op = DveOp("CUBE_MUL_ANT", spec, subdim=False, uops_sha=shas)
    dve_ops.OPS.append(op)
    dve_ops._SUB_OPCODE_FOR_NAME[op.name] = row
    dve_ops.CUSTOM_DVE_SPECS[op.name] = spec
    _CUBE_OP = op
    return op


# ---------------- device program ----------------
_PROGRAM_CACHE = {}

f16 = mybir.dt.float16
f32 = mybir.dt.float32

# patch chunks: (col0, row0, nrows)
CHUNKS = []
_r = 0
while _r < OH:
    nr = min(ROWS_PER_CHUNK, OH - _r)
    CHUNKS.append((_r * OH, _r, nr))
    _r += nr
NCH = len(CHUNKS)  # 8
NPIXCH = NPIX // PIXCH  # 4


def _build_program():
    key = "v2"
    if key in _PROGRAM_CACHE:
        return _PROGRAM_CACHE[key]

    nc = bacc.Bacc("TRN2", target_bir_lowering=False, debug=False)
    x_d = nc.dram_tensor("xin", [8, NPIX], f16, kind="ExternalInput").ap()
    wm_d = nc.dram_tensor("wm", [128, WCOLS], f16, kind="ExternalInput").ap()
    bias_d = nc.dram_tensor("bias32", [128, 2], f32, kind="ExternalInput").ap()
    z_d = nc.dram_tensor("zout", [16, P], f16, kind="ExternalOutput").ap()

    CUBE = _register_cube_mul()
    SIN = mybir.ActivationFunctionType.Sin
    COPY = mybir.ActivationFunctionType.Copy
    SQUARE = mybir.ActivationFunctionType.Square

    with tile.TileContext(nc) as tc:
        from contextlib import ExitStack

        with ExitStack() as ctx:
            const_pool = ctx.enter_context(tc.tile_pool(name="const", bufs=1))
            v_pool = ctx.enter_context(tc.tile_pool(name="v", bufs=2))
            ysb_pool = ctx.enter_context(tc.tile_pool(name="ysb", bufs=4))
            m_pool = ctx.enter_context(tc.tile_pool(name="m", bufs=4))
            sq_pool = ctx.enter_context(tc.tile_pool(name="sq", bufs=2))
            zs_pool = ctx.enter_context(tc.tile_pool(name="zs", bufs=2))
            aps_pool = ctx.enter_context(
                tc.tile_pool(name="aps", bufs=1, space="PSUM")
            )
            yps = [
                ctx.enter_context(tc.tile_pool(name=f"y{i}", bufs=1, space="PSUM"))
                for i in range(N_CHAINS)
            ]

            wm_sb = const_pool.tile([128, WCOLS], f16)
            x_sb = const_pool.tile([8, NPIX], f16)
            bias_sb = const_pool.tile([128, 2], f32)
            ones_sb = const_pool.tile([128, PIXCH], f16)
            # Phi pixel arrays: plane 0 = -cos(alpha)/4, plane 1 = -sin/4
            phi = const_pool.tile([128, 2, HW, HW], f16)
            zbuf = const_pool.tile([16, P], f16)

            def wma(c0, n=128):
                return wm_sb[:, c0 : c0 + n]

            chain = {}

            def do_init(_a, _b):
                nc.gpsimd.dma_start(bias_sb[:], bias_d[:])
                nc.vector.memset(ones_sb[:], 1.0)

            def do_wm_dma(i, _p):
                # alpha cols first on the sync queue (critical path);
                # the rest via gpsimd SWDGE in parallel
                if i == 0:
                    nc.scalar.dma_start(wm_sb[:, 0:128], wm_d[:, 0:128])
                else:
                    nc.gpsimd.dma_start(
                        wm_sb[:, 128:WCOLS], wm_d[:, 128:WCOLS]
                    )

            def do_xdma(i, _p):
                nc.sync.dma_start(
                    x_sb[:, i * PIXCH : (i + 1) * PIXCH],
                    x_d[:, i * PIXCH : (i + 1) * PIXCH],
                )

            def do_pix(i, _p):
                a_ps = aps_pool.tile([128, PIXCH], f32, tag="aps", name=f"a{i}")
                for h in range(PIXCH // 512):
                    nc.tensor.matmul(
                        a_ps[:, h * 512 : (h + 1) * 512],
                        wm_sb[0:8, A_ALPHA : A_ALPHA + 128],
                        x_sb[:, i * PIXCH + h * 512 : i * PIXCH + (h + 1) * 512],
                        start=True,
                        stop=True,
                    )
                vc = v_pool.tile([128, PIXCH], f16, tag="vc", name=f"vc{i}")
                vs = v_pool.tile([128, PIXCH], f16, tag="vs", name=f"vs{i}")
                nc.scalar.activation(vc[:], a_ps[:], SIN, bias=bias_sb[:, 0:1])
                nc.scalar.activation(vs[:], a_ps[:], SIN, bias=bias_sb[:, 1:2])
                r = i * (PIXCH // HW)
                nrr = PIXCH // HW
                nc.vector._custom_dve(
                    CUBE,
                    out=phi[:, 0, r : r + nrr, :],
                    in0=vc[:],
                    in1=ones_sb[:],
                    s0=0.75,
                )
                nc.vector._custom_dve(
                    CUBE,
                    out=phi[:, 1, r : r + nrr, :],
                    in0=vs[:],
                    in1=ones_sb[:],
                    s0=0.75,
                )

            def do_pos(ci, p):
                c0, r0, nr = CHUNKS[ci]
                C = nr * OH
                dy, dx = p // 3, p % 3
                ypool = yps[ci % N_CHAINS]
                y_new = ypool.tile(
                    [128, ROWS_PER_CHUNK, OH],
                    f32,
                    tag=f"y{ci % N_CHAINS}",
                    name=f"y{ci}_{p}",
                )
                if p == 0:
                    rhsC = phi[:, 0, r0 + dy : r0 + dy + nr, dx : dx + OH]
                    rhsS = phi[:, 1, r0 + dy : r0 + dy + nr, dx : dx + OH]
                    nc.tensor.matmul(
                        y_new[:, :nr, :].opt(), wma(A_P0C), rhsC, start=True,
                        stop=False,
                    )
                    nc.tensor.matmul(
                        y_new[:, :nr, :].opt(), wma(A_P0S), rhsS, start=False,
                        stop=True,
                    )
                else:
                    y_prev = chain[ci]
                    m = m_pool.tile(
                        [128, 2, ROWS_PER_CHUNK, OH], f16, tag="m", name=f"m{ci}_{p}"
                    )
                    in0 = phi[:, :, r0 + dy : r0 + dy + nr, dx : dx + OH]
                    if p in DVE_CV_POS:
                        # fused: DVE reads the fp32 state straight from PSUM
                        # (1x mode) - no separate downconvert op
                        in1 = (
                            y_prev[:, :nr, :]
                            .unsqueeze(1)
                            .broadcast_to([128, 2, nr, OH])
                        )
                        nc.vector.tensor_mul(m[:, :, :nr, :], in0, in1)
                    else:
                        ysb = ysb_pool.tile(
                            [128, ROWS_PER_CHUNK, OH],
                            f16,
                            tag="ysb",
                            name=f"ysb{ci}_{p}",
                        )
                        nc.scalar.activation(
                            ysb[:, :nr, :].opt(), y_prev[:, :nr, :].opt(), COPY
                        )
                        in1 = (
                            ysb[:, :nr, :]
                            .unsqueeze(1)
                            .broadcast_to([128, 2, nr, OH])
                        )
                        if p in POOL_TT_POS:
                            nc.gpsimd.tensor_mul(m[:, :, :nr, :], in0, in1)
                        else:
                            nc.vector.tensor_mul(m[:, :, :nr, :], in0, in1)
                    col = A_M + (p - 1) * 256
                    nc.tensor.matmul(
                        y_new[:, :nr, :].opt(),
                        wma(col),
                        m[:, 0, :nr, :].opt(),
                        start=True,
                        stop=False,
                    )
                    nc.tensor.matmul(
                        y_new[:, :nr, :].opt(),
                        wma(col + 128),
                        m[:, 1, :nr, :].opt(),
                        start=False,
                        stop=True,
                    )
                chain[ci] = y_new

            def do_meas(ci, _p):
                c0, r0, nr = CHUNKS[ci]
                C = nr * OH
                y9 = chain.pop(ci)
                sq = sq_pool.tile(
                    [128, ROWS_PER_CHUNK, OH], f16, tag="sq", name=f"sq{ci}"
                )
                nc.scalar.activation(
                    sq[:, :nr, :].opt(), y9[:, :nr, :].opt(), SQUARE
                )
                # zq reuses the finished chain's PSUM bank
                ypool = yps[ci % N_CHAINS]
                zq = ypool.tile(
                    [16, 512], f32, tag=f"y{ci % N_CHAINS}", name=f"zq{ci}"
                )
                nc.tensor.matmul(
                    zq[:, :C],
                    wma(A_Z, 16),
                    sq[:, :nr, :].opt(),
                    start=True,
                    stop=True,
                )
                nc.scalar.activation(zbuf[:, c0 : c0 + C], zq[:, :C], COPY)
                if ci == NCH - 1:
                    nc.sync.dma_start(z_d[:], zbuf[:])

            # time-ordered emission; pixel chunks lead their consumers
            OFF = 1.45
            events = [(-11.0, 0, 0, 0, do_init)]
            events.append((-10.5, 0, 0, 0, do_wm_dma))
            events.append((-10.4, 0, 1, 0, do_wm_dma))
            for j in range(NPIXCH):
                t = max(-8.0, (j - 1) * OFF - 3.0)
                events.append((t - 0.4, 0, j, 0, do_xdma))
                events.append((t, 1, j, 0, do_pix))
            for ci in range(NCH):
                for p in range(NPOS):
                    events.append((ci * OFF + p, 2, ci, p, do_pos))
                events.append((ci * OFF + NPOS, 3, ci, 0, do_meas))
            events.sort(key=lambda e: (e[0], e[1], e[2]))
            for _t, _k, ci, p, fn in events:
                fn(ci, p)

    nc.compile()
    _PROGRAM_CACHE[key] = nc
    return nc


# ---------------- entry point ----------------
_BIAS32 = np.zeros((128, 2), dtype=np.float32)
_BIAS32[:, 0] = np.pi / 6


def kernel(x, weights):
    x = np.asarray(x, dtype=np.float32)
    weights = np.asarray(weights, dtype=np.float32)
    wm = _build_wm(weights)

    nc = _build_program()
    in_maps = []
    for c in range(N_CORES):
        xc = np.ascontiguousarray(
            x[2 * c : 2 * c + 2].reshape(8, NPIX), dtype=np.float16
        )
        in_maps.append({"xin": xc, "wm": wm, "bias32": _BIAS32})
    res = run_bass_kernel_spmd(nc, in_maps, list(range(N_CORES)))

    out = np.zeros((B, 2 * IN_CH, OH, OH), dtype=np.float32)
    for c in range(N_CORES):
        z = np.asarray(res.results[c]["zout"]).astype(np.float32)  # (16, P)
        for k in range(2):
            for img in range(2):
                b = 2 * c + img
                for q in range(IN_CH):
                    out[b, k * IN_CH + q] = z[k * 8 + img * 4 + q].reshape(OH, OH)
    return out


# revision 13
# speedup vs baseline: 1.1182x; 1.1182x over previous
"""Trainium2 Bass kernel for nn_ChannelWiseQuantumKernel (v2).

Method (per-pixel phase precompute + compact z/w state recurrence):

The per-position data RY gates are diagonal phase gates in the (SH)-transformed
basis: the 16 phase angles per patch-position are SHIFTED READS of a per-PIXEL
field alpha_j(img, h, w) = sum_ch +-theta_ch/2 (a 4->16 signed channel mix).
So all trig is computed once per pixel instead of once per patch-position
(9x less work): PhiC = -cos(alpha)/4, PhiS = -sin(alpha)/4 stored as fp16
pixel arrays [128, 2, 64, 64] (partitions = kernel(2) x img(2) x state(32)).

State: compact 32 real values per (kernel, img): y = [yR(16); yI(16)], all
four (k, img) groups packed in 128 partitions. Per position p:
    z = PhiC_view(p) * y ;  w = PhiS_view(p) * y      (one fused DVE TT, 2x
                                                       fp16 mode, strided
                                                       shifted-window views)
    y' = M1_p @ z + M2_p @ w                          (two accumulated fp16
                                                       128x128 matmuls)
with M1 = -4*[[Gr,-Gi],[Gi,Gr]], M2 = -4*[[-Gi,-Gr],[Gr,-Gi]] per kernel.
A ScalarE copy downconverts y (fp32 PSUM) to fp16 SBUF each position so the
DVE multiply runs in the packed 2x mode. Measurement: Square + one
128->16 signed-sum matmul; 9 positions -> <Z_q> for 2 kernels x 2 images.
Patches stream in 8 column-chunks of 8 output rows (496 cols) per core.
"""

import sys

sys.path.insert(0, "/opt/trn_rl_repo")

import numpy as np

import concourse.bacc as bacc
import concourse.bass as bass
import concourse.tile as tile
from concourse import mybir
from concourse.bass_utils import run_bass_kernel_spmd

# ---------------- problem constants ----------------
IN_CH = 4
KSZ = 3
NPOS = 9
DIM = 16
B = 16
HW = 64
OH = HW - KSZ + 1  # 62
P = OH * OH  # 3844 patches per image
N_CORES = 8
NPIX = HW * HW  # 4096
PIXCH = 512  # pixels per pixel-chunk (8 rows of 64)
ROWS_PER_CHUNK = 8  # output rows per patch-chunk
N_CHAINS = 7  # patch-chunk chains in flight (1 PSUM bank each)
DVE_CV_POS = (3, 6)  # positions with fused direct-PSUM multiply on VectorE
POOL_TT_POS = (2, 5)  # positions whose phase-multiply runs on GpSimd

# wm (fp16 weight matrix) column layout
A_ALPHA = 0
A_P0C = 128
A_P0S = 256
A_M = 384  # 16 x 128 cols: (p-1)*256 + {0 (M1), 128 (M2)}
A_Z = A_M + 16 * 128  # 2432
WCOLS = A_Z + 16  # 2448

# ---------------- host-side constant math (weights-only, O(1)) ----------------
_H = np.array([[1, 1], [1, -1]], dtype=np.complex128) / np.sqrt(2)
_S = np.array([[1, 0], [0, 1j]], dtype=np.complex128)
_A1 = _S @ _H


def _kron_n(mats):
    out = np.array([[1.0 + 0j]])
    for m in mats:
        out = np.kron(out, m)
    return out


_AA = _kron_n([_A1] * IN_CH)
_U1 = _A1.conj().T @ (np.array([1.0, 1.0]) / np.sqrt(2))
_YINIT = _kron_n([_U1.reshape(2, 1)] * IN_CH).reshape(DIM)

_SGN = np.array(
    [[2 * ((i >> (3 - ch)) & 1) - 1 for i in range(DIM)] for ch in range(IN_CH)],
    dtype=np.float64,
)
_SIGMA = np.array(
    [[1 - 2 * ((i >> (3 - q)) & 1) for i in range(DIM)] for q in range(IN_CH)],
    dtype=np.float64,
)


def _rx(t):
    c, s = np.cos(t / 2), np.sin(t / 2)
    return np.array([[c, -1j * s], [-1j * s, c]])


def _ry(t):
    c, s = np.cos(t / 2), np.sin(t / 2)
    return np.array([[c, -s], [s, c]])


def _rz(t):
    e = np.exp(-0.5j * t)
    return np.array([[e, 0], [0, np.conj(e)]])


def _embed(U, q):
    mats = [np.eye(2, dtype=complex)] * IN_CH
    mats[q] = U
    return _kron_n(mats)


def _cx(cq, tq):
    M = np.zeros((DIM, DIM), dtype=complex)
    for i in range(DIM):
        bits = [(i >> (3 - q)) & 1 for q in range(4)]
        j = i
        if bits[cq] == 1:
            bits2 = bits.copy()
            bits2[tq] ^= 1
            j = sum(b << (3 - q) for q, b in enumerate(bits2))
        M[j, i] = 1
    return M


def _build_G(w_flat):
    w = np.float64(w_flat).reshape(NPOS, 1, IN_CH, 3)
    Gs = []
    for pos in range(NPOS):
        U = np.eye(DIM, dtype=complex)
        for q in range(IN_CH):
            R = _rz(w[pos, 0, q, 2]) @ _ry(w[pos, 0, q, 1]) @ _rx(w[pos, 0, q, 0])
            U = _embed(R, q) @ U
        for q in range(IN_CH - 1):
            U = _cx(q, q + 1) @ U
        U = _cx(IN_CH - 1, 0) @ U
        Gs.append(_AA.conj().T @ U @ _AA)
    G8f = _AA @ Gs[8]
    return Gs, G8f


def _build_wm(weights):
    """All matmul lhsT constants, fp16 [128, WCOLS].

    Partition groups g = k*2 + img occupy partitions [32g, 32g+32)."""
    wm = np.zeros((128, WCOLS), dtype=np.float64)
    # alpha lhsT: rows (img*4+ch) 0..8, cols (g, j32); emits alpha/3 directly
    for k in range(2):
        for img in range(2):
            g = k * 2 + img
            for ch in range(IN_CH):
                for j in range(32):
                    wm[img * 4 + ch, A_ALPHA + g * 32 + j] = _SGN[ch, j % 16] / 6.0
    for k in range(2):
        Gs, G8f = _build_G(weights[k])
        G0c = Gs[0] @ np.diag(_YINIT)
        Pm, Qm = G0c.real, G0c.imag
        Ac = np.vstack([Pm, Qm])  # 32x16: c16 -> [y1R; y1I]
        As = np.vstack([-Qm, Pm])
        for img in range(2):
            g = k * 2 + img
            r0 = g * 32
            # pos0: lhsT[r0+j, A_P0C + r0+o] = -4*Ac[o, j] (j<16)
            wm[r0 : r0 + 16, A_P0C + r0 : A_P0C + r0 + 32] = -4.0 * Ac.T
            wm[r0 : r0 + 16, A_P0S + r0 : A_P0S + r0 + 32] = -4.0 * As.T
        for p in range(1, 9):
            G = G8f if p == 8 else Gs[p]
            Gr, Gi = G.real, G.imag
            M1 = np.block([[Gr, -Gi], [Gi, Gr]])
            M2 = np.block([[-Gi, -Gr], [Gr, -Gi]])
            col = A_M + (p - 1) * 256
            for img in range(2):
                g = k * 2 + img
                r0 = g * 32
                wm[r0 : r0 + 32, col + r0 : col + r0 + 32] = -4.0 * M1.T
                wm[r0 : r0 + 32, col + 128 + r0 : col + 128 + r0 + 32] = -4.0 * M2.T
        # Z lhsT: out row r = k*8 + img*4 + q
        for img in range(2):
            g = k * 2 + img
            r0 = g * 32
            for q in range(IN_CH):
                for j in range(32):
                    wm[r0 + j, A_Z + k * 8 + img * 4 + q] = _SIGMA[q, j % 16]
    return wm.astype(np.float16)


# ---------------- custom fused DVE op: out = ((v^2 - 3/4) * v) * y ----------
_CUBE_OP = None


def _register_cube_mul():
    """Register the fused triple-angle multiply as a custom DVE op.

    One VectorE instruction computes ((v*v - c0) * v) * y, which applies the
    full range-reduced phase factor to the state in a single pass."""
    global _CUBE_OP
    if _CUBE_OP is not None:
        return _CUBE_OP
    import concourse.dve_ops as dve_ops

    for o in dve_ops.OPS:
        if o.name == "CUBE_MUL_ANT":
            _CUBE_OP = o
            return o
    from concourse.dve_ops import DveOp
    from concourse.dve_spec import C0, Spec, Src0, Src1, lower
    from concourse.dve_uop import DveOpSpec

    body = ((Src0 * Src0 - C0) * Src0) * Src1
    spec = Spec(
        body=body,
        reference=lambda in0, in1, c0, c1, c2: (
            ((in0.astype(np.float32) * in0 - c0) * in0) * in1
        ),
    )
    row = max(dve_ops._SUB_OPCODE_FOR_NAME.values()) + 1
    shas = {}
    for ver in ("v3", "v4"):
        uops = lower(spec, ver=ver)
        shas[ver] = DveOpSpec(
            name="CUBE_MUL_ANT", opcode=row, uops=uops, rd1_en=True
        ).sha(ver)
    op = DveOp("CUBE_MUL_ANT", spec, subdim=False, uops_sha=shas)
    dve_ops.OPS.append(op)
    dve_ops._SUB_OPCODE_FOR_NAME[op.name] = row
    dve_ops.CUSTOM_DVE_SPECS[op.name] = spec
    _CUBE_OP = op
    return op


# ---------------- device program ----------------
_PROGRAM_CACHE = {}

f16 = mybir.dt.float16
f32 = mybir.dt.float32

# patch chunks: (col0, row0, nrows)
CHUNKS = []
_r = 0
while _r < OH:
    nr = min(ROWS_PER_CHUNK, OH - _r)
    CHUNKS.append((_r * OH, _r, nr))
    _r += nr
NCH = len(CHUNKS)  # 8
NPIXCH = NPIX // PIXCH  # 4


def _build_program():
    key = "v2"
    if key in _PROGRAM_CACHE:
        return _PROGRAM_CACHE[key]

    nc = bacc.Bacc("TRN2", target_bir_lowering=False, debug=False)
    x_d = nc.dram_tensor("xin", [8, NPIX], f16, kind="ExternalInput").ap()
    wm_d = nc.dram_tensor("wm", [128, WCOLS], f16, kind="ExternalInput").ap()
    bias_d = nc.dram_tensor("bias32", [128, 2], f32, kind="ExternalInput").ap()
    z_d = nc.dram_tensor("zout", [16, P], f16, kind="ExternalOutput").ap()

    CUBE = _register_cube_mul()
    SIN = mybir.ActivationFunctionType.Sin
    COPY = mybir.ActivationFunctionType.Copy
    SQUARE = mybir.ActivationFunctionType.Square

    with tile.TileContext(nc) as tc:
        from contextlib import ExitStack

        with ExitStack() as ctx:
            const_pool = ctx.enter_context(tc.tile_pool(name="const", bufs=1))
            v_pool = ctx.enter_context(tc.tile_pool(name="v", bufs=2))
            ysb_pool = ctx.enter_context(tc.tile_pool(name="ysb", bufs=4))
            m_pool = ctx.enter_context(tc.tile_pool(name="m", bufs=4))
            sq_pool = ctx.enter_context(tc.tile_pool(name="sq", bufs=2))
            zs_pool = ctx.enter_context(tc.tile_pool(name="zs", bufs=2))
            aps_pool = ctx.enter_context(
                tc.tile_pool(name="aps", bufs=1, space="PSUM")
            )
            yps = [
                ctx.enter_context(tc.tile_pool(name=f"y{i}", bufs=1, space="PSUM"))
                for i in range(N_CHAINS)
            ]

            wm_sb = const_pool.tile([128, WCOLS], f16)
            x_sb = const_pool.tile([8, NPIX], f16)
            bias_sb = const_pool.tile([128, 2], f32)
            ones_sb = const_pool.tile([128, PIXCH], f16)
            # Phi pixel arrays: plane 0 = -cos(alpha)/4, plane 1 = -sin/4
            phi = const_pool.tile([128, 2, HW, HW], f16)
            zbuf = const_pool.tile([16, P], f16)

            def wma(c0, n=128):
                return wm_sb[:, c0 : c0 + n]

            chain = {}

            def do_init(_a, _b):
                nc.gpsimd.dma_start(bias_sb[:], bias_d[:])
                nc.vector.memset(ones_sb[:], 1.0)

            def do_wm_dma(i, _p):
                # alpha cols first on the sync queue (critical path);
                # the rest via gpsimd SWDGE in parallel
                if i == 0:
                    nc.scalar.dma_start(wm_sb[:, 0:128], wm_d[:, 0:128])
                else:
                    nc.gpsimd.dma_start(
                        wm_sb[:, 128:WCOLS], wm_d[:, 128:WCOLS]
                    )

            def do_xdma(i, _p):
                nc.sync.dma_start(
                    x_sb[:, i * PIXCH : (i + 1) * PIXCH],
                    x_d[:, i * PIXCH : (i + 1) * PIXCH],
                )

            def do_pix(i, _p):
                a_ps = aps_pool.tile([128, PIXCH], f32, tag="aps", name=f"a{i}")
                for h in range(PIXCH // 512):
                    nc.tensor.matmul(
                        a_ps[:, h * 512 : (h + 1) * 512],
                        wm_sb[0:8, A_ALPHA : A_ALPHA + 128],
                        x_sb[:, i * PIXCH + h * 512 : i * PIXCH + (h + 1) * 512],
                        start=True,
                        stop=True,
                    )
                vc = v_pool.tile([128, PIXCH], f16, tag="vc", name=f"vc{i}")
                vs = v_pool.tile([128, PIXCH], f16, tag="vs", name=f"vs{i}")
                nc.scalar.activation(vc[:], a_ps[:], SIN, bias=bias_sb[:, 0:1])
                nc.scalar.activation(vs[:], a_ps[:], SIN, bias=bias_sb[:, 1:2])
                r = i * (PIXCH // HW)
                nrr = PIXCH // HW
                nc.vector._custom_dve(
                    CUBE,
                    out=phi[:, 0, r : r + nrr, :],
                    in0=vc[:],
                    in1=ones_sb[:],
                    s0=0.75,
                )
                nc.vector._custom_dve(
                    CUBE,
                    out=phi[:, 1, r : r + nrr, :],
                    in0=vs[:],
                    in1=ones_sb[:],
                    s0=0.75,
                )

            def do_pos(ci, p):
                c0, r0, nr = CHUNKS[ci]
                C = nr * OH
                dy, dx = p // 3, p % 3
                ypool = yps[ci % N_CHAINS]
                y_new = ypool.tile(
                    [128, ROWS_PER_CHUNK, OH],
                    f32,
                    tag=f"y{ci % N_CHAINS}",
                    name=f"y{ci}_{p}",
                )
                if p == 0:
                    rhsC = phi[:, 0, r0 + dy : r0 + dy + nr, dx : dx + OH]
                    rhsS = phi[:, 1, r0 + dy : r0 + dy + nr, dx : dx + OH]
                    nc.tensor.matmul(
                        y_new[:, :nr, :].opt(), wma(A_P0C), rhsC, start=True,
                        stop=False,
                    )
                    nc.tensor.matmul(
                        y_new[:, :nr, :].opt(), wma(A_P0S), rhsS, start=False,
                        stop=True,
                    )
                else:
                    y_prev = chain[ci]
                    m = m_pool.tile(
                        [128, 2, ROWS_PER_CHUNK, OH], f16, tag="m", name=f"m{ci}_{p}"
                    )
                    in0 = phi[:, :, r0 + dy : r0 + dy + nr, dx : dx + OH]
                    if p in DVE_CV_POS:
                        # fused: DVE reads the fp32 state straight from PSUM
                        # (1x mode) - no separate downconvert op
                        in1 = (
                            y_prev[:, :nr, :]
                            .unsqueeze(1)
                            .broadcast_to([128, 2, nr, OH])
                        )
                        nc.vector.tensor_mul(m[:, :, :nr, :], in0, in1)
                    else:
                        ysb = ysb_pool.tile(
                            [128, ROWS_PER_CHUNK, OH],
                            f16,
                            tag="ysb",
                            name=f"ysb{ci}_{p}",
                        )
                        nc.scalar.activation(
                            ysb[:, :nr, :].opt(), y_prev[:, :nr, :].opt(), COPY
                        )
                        in1 = (
                            ysb[:, :nr, :]
                            .unsqueeze(1)
                            .broadcast_to([128, 2, nr, OH])
                        )
                        if p in POOL_TT_POS:
                            nc.gpsimd.tensor_mul(m[:, :, :nr, :], in0, in1)
                        else:
                            nc.vector.tensor_mul(m[:, :, :nr, :], in0, in1)
                    col = A_M + (p - 1) * 256
                    nc.tensor.matmul(
                        y_new[:, :nr, :].opt(),
                        wma(col),
                        m[:, 0, :nr, :].opt(),
                        start=True,
                        stop=False,
                    )
                    nc.tensor.matmul(
                        y_new[:, :nr, :].opt(),
                        wma(col + 128),
                        m[:, 1, :nr, :].opt(),
                        start=False,
                        stop=True,
                    )
                chain[ci] = y_new

            def do_meas(ci, _p):
                c0, r0, nr = CHUNKS[ci]
                C = nr * OH
                y9 = chain.pop(ci)
                sq = sq_pool.tile(
                    [128, ROWS_PER_CHUNK, OH], f16, tag="sq", name=f"sq{ci}"
                )
                nc.scalar.activation(
                    sq[:, :nr, :].opt(), y9[:, :nr, :].opt(), SQUARE
                )
                # zq reuses the finished chain's PSUM bank
                ypool = yps[ci % N_CHAINS]
                zq = ypool.tile(
                    [16, 512], f32, tag=f"y{ci % N_CHAINS}", name=f"zq{ci}"
                )
                nc.tensor.matmul(
                    zq[:, :C],
                    wma(A_Z, 16),
                    sq[:, :nr, :].opt(),
                    start=True,
                    stop=True,
                )
                nc.scalar.activation(zbuf[:, c0 : c0 + C], zq[:, :C], COPY)
                if ci == NCH - 1:
                    nc.sync.dma_start(z_d[:], zbuf[:])

            # time-ordered emission; pixel chunks lead their consumers
            OFF = 1.45
            events = [(-11.0, 0, 0, 0, do_init)]
            events.append((-10.5, 0, 0, 0, do_wm_dma))
            events.append((-10.4, 0, 1, 0, do_wm_dma))
            for j in range(NPIXCH):
                t = max(-8.0, (j - 1) * OFF - 3.0)
                events.append((t - 0.4, 0, j, 0, do_xdma))
                events.append((t, 1, j, 0, do_pix))
            for ci in range(NCH):
                for p in range(NPOS):
                    events.append((ci * OFF + p, 2, ci, p, do_pos))
                events.append((ci * OFF + NPOS, 3, ci, 0, do_meas))
            events.sort(key=lambda e: (e[0], e[1], e[2]))
            for _t, _k, ci, p, fn in events:
                fn(ci, p)

    nc.compile()
    _PROGRAM_CACHE[key] = nc
    return nc


# ---------------- entry point ----------------
_BIAS32 = np.zeros((128, 2), dtype=np.float32)
_BIAS32[:, 0] = np.pi / 6


def kernel(x, weights):
    x = np.asarray(x, dtype=np.float32)
    weights = np.asarray(weights, dtype=np.float32)
    wm = _build_wm(weights)

    nc = _build_program()
    in_maps = []
    for c in range(N_CORES):
        xc = np.ascontiguousarray(
            x[2 * c : 2 * c + 2].reshape(8, NPIX), dtype=np.float16
        )
        in_maps.append({"xin": xc, "wm": wm, "bias32": _BIAS32})
    res = run_bass_kernel_spmd(nc, in_maps, list(range(N_CORES)))

    out = np.zeros((B, 2 * IN_CH, OH, OH), dtype=np.float32)
    for c in range(N_CORES):
        z = np.asarray(res.results[c]["zout"]).astype(np.float32)  # (16, P)
        for k in range(2):
            for img in range(2):
                b = 2 * c + img
                for q in range(IN_CH):
                    out[b, k * IN_CH + q] = z[k * 8 + img * 4 + q].reshape(OH, OH)
    return out


# revision 14
# speedup vs baseline: 1.1516x; 1.0299x over previous
"""Trainium2 Bass kernel for nn_ChannelWiseQuantumKernel (v2).

Method (per-pixel phase precompute + compact z/w state recurrence):

The per-position data RY gates are diagonal phase gates in the (SH)-transformed
basis: the 16 phase angles per patch-position are SHIFTED READS of a per-PIXEL
field alpha_j(img, h, w) = sum_ch +-theta_ch/2 (a 4->16 signed channel mix).
So all trig is computed once per pixel instead of once per patch-position
(9x less work): PhiC = -cos(alpha)/4, PhiS = -sin(alpha)/4 stored as fp16
pixel arrays [128, 2, 64, 64] (partitions = kernel(2) x img(2) x state(32)).

State: compact 32 real values per (kernel, img): y = [yR(16); yI(16)], all
four (k, img) groups packed in 128 partitions. Per position p:
    z = PhiC_view(p) * y ;  w = PhiS_view(p) * y      (one fused DVE TT, 2x
                                                       fp16 mode, strided
                                                       shifted-window views)
    y' = M1_p @ z + M2_p @ w                          (two accumulated fp16
                                                       128x128 matmuls)
with M1 = -4*[[Gr,-Gi],[Gi,Gr]], M2 = -4*[[-Gi,-Gr],[Gr,-Gi]] per kernel.
A ScalarE copy downconverts y (fp32 PSUM) to fp16 SBUF each position so the
DVE multiply runs in the packed 2x mode. Measurement: Square + one
128->16 signed-sum matmul; 9 positions -> <Z_q> for 2 kernels x 2 images.
Patches stream in 8 column-chunks of 8 output rows (496 cols) per core.
"""

import sys

sys.path.insert(0, "/opt/trn_rl_repo")

import numpy as np

import concourse.bacc as bacc
import concourse.bass as bass
import concourse.tile as tile
from concourse import mybir
from concourse.bass_utils import run_bass_kernel_spmd

# ---------------- problem constants ----------------
IN_CH = 4
KSZ = 3
NPOS = 9
DIM = 16
B = 16
HW = 64
OH = HW - KSZ + 1  # 62
P = OH * OH  # 3844 patches per image
N_CORES = 8
NPIX = HW * HW  # 4096
PIXCH = 512  # pixels per pixel-chunk (8 rows of 64)
ROWS_PER_CHUNK = 8  # output rows per patch-chunk
N_CHAINS = 7  # patch-chunk chains in flight (1 PSUM bank each)
DVE_CV_POS = (3, 6)  # positions with fused direct-PSUM multiply on VectorE
POOL_TT_POS = (2, 5)  # positions whose phase-multiply runs on GpSimd

# wm (fp16 weight matrix) column layout
A_ALPHA = 0
A_P0C = 128
A_P0S = 256
A_M = 384  # 16 x 128 cols: (p-1)*256 + {0 (M1), 128 (M2)}
A_Z = A_M + 16 * 128  # 2432
WCOLS = A_Z + 16  # 2448

# ---------------- host-side constant math (weights-only, O(1)) ----------------
_H = np.array([[1, 1], [1, -1]], dtype=np.complex128) / np.sqrt(2)
_S = np.array([[1, 0], [0, 1j]], dtype=np.complex128)
_A1 = _S @ _H


def _kron_n(mats):
    out = np.array([[1.0 + 0j]])
    for m in mats:
        out = np.kron(out, m)
    return out


_AA = _kron_n([_A1] * IN_CH)
_U1 = _A1.conj().T @ (np.array([1.0, 1.0]) / np.sqrt(2))
_YINIT = _kron_n([_U1.reshape(2, 1)] * IN_CH).reshape(DIM)

_SGN = np.array(
    [[2 * ((i >> (3 - ch)) & 1) - 1 for i in range(DIM)] for ch in range(IN_CH)],
    dtype=np.float64,
)
_SIGMA = np.array(
    [[1 - 2 * ((i >> (3 - q)) & 1) for i in range(DIM)] for q in range(IN_CH)],
    dtype=np.float64,
)


def _rx(t):
    c, s = np.cos(t / 2), np.sin(t / 2)
    return np.array([[c, -1j * s], [-1j * s, c]])


def _ry(t):
    c, s = np.cos(t / 2), np.sin(t / 2)
    return np.array([[c, -s], [s, c]])


def _rz(t):
    e = np.exp(-0.5j * t)
    return np.array([[e, 0], [0, np.conj(e)]])


def _embed(U, q):
    mats = [np.eye(2, dtype=complex)] * IN_CH
    mats[q] = U
    return _kron_n(mats)


def _cx(cq, tq):
    M = np.zeros((DIM, DIM), dtype=complex)
    for i in range(DIM):
        bits = [(i >> (3 - q)) & 1 for q in range(4)]
        j = i
        if bits[cq] == 1:
            bits2 = bits.copy()
            bits2[tq] ^= 1
            j = sum(b << (3 - q) for q, b in enumerate(bits2))
        M[j, i] = 1
    return M


def _build_G(w_flat):
    w = np.float64(w_flat).reshape(NPOS, 1, IN_CH, 3)
    Gs = []
    for pos in range(NPOS):
        U = np.eye(DIM, dtype=complex)
        for q in range(IN_CH):
            R = _rz(w[pos, 0, q, 2]) @ _ry(w[pos, 0, q, 1]) @ _rx(w[pos, 0, q, 0])
            U = _embed(R, q) @ U
        for q in range(IN_CH - 1):
            U = _cx(q, q + 1) @ U
        U = _cx(IN_CH - 1, 0) @ U
        Gs.append(_AA.conj().T @ U @ _AA)
    G8f = _AA @ Gs[8]
    return Gs, G8f


def _build_wm(weights):
    """All matmul lhsT constants, fp16 [128, WCOLS].

    Partition groups g = k*2 + img occupy partitions [32g, 32g+32)."""
    wm = np.zeros((128, WCOLS), dtype=np.float64)
    # alpha lhsT: rows (img*4+ch) 0..8, cols (g, j32); emits alpha/3 directly
    for k in range(2):
        for img in range(2):
            g = k * 2 + img
            for ch in range(IN_CH):
                for j in range(32):
                    wm[img * 4 + ch, A_ALPHA + g * 32 + j] = _SGN[ch, j % 16] / 6.0
    for k in range(2):
        Gs, G8f = _build_G(weights[k])
        G0c = Gs[0] @ np.diag(_YINIT)
        Pm, Qm = G0c.real, G0c.imag
        Ac = np.vstack([Pm, Qm])  # 32x16: c16 -> [y1R; y1I]
        As = np.vstack([-Qm, Pm])
        for img in range(2):
            g = k * 2 + img
            r0 = g * 32
            # pos0: lhsT[r0+j, A_P0C + r0+o] = -4*Ac[o, j] (j<16)
            wm[r0 : r0 + 16, A_P0C + r0 : A_P0C + r0 + 32] = -4.0 * Ac.T
            wm[r0 : r0 + 16, A_P0S + r0 : A_P0S + r0 + 32] = -4.0 * As.T
        for p in range(1, 9):
            G = G8f if p == 8 else Gs[p]
            Gr, Gi = G.real, G.imag
            M1 = np.block([[Gr, -Gi], [Gi, Gr]])
            M2 = np.block([[-Gi, -Gr], [Gr, -Gi]])
            col = A_M + (p - 1) * 256
            for img in range(2):
                g = k * 2 + img
                r0 = g * 32
                wm[r0 : r0 + 32, col + r0 : col + r0 + 32] = -4.0 * M1.T
                wm[r0 : r0 + 32, col + 128 + r0 : col + 128 + r0 + 32] = -4.0 * M2.T
        # Z lhsT: out row r = k*8 + img*4 + q
        for img in range(2):
            g = k * 2 + img
            r0 = g * 32
            for q in range(IN_CH):
                for j in range(32):
                    wm[r0 + j, A_Z + k * 8 + img * 4 + q] = _SIGMA[q, j % 16]
    return wm.astype(np.float16)


# ---------------- custom fused DVE op: out = ((v^2 - 3/4) * v) * y ----------
_CUBE_OP = None


def _register_cube_mul():
    """Register the fused triple-angle multiply as a custom DVE op.

    One VectorE instruction computes ((v*v - c0) * v) * y, which applies the
    full range-reduced phase factor to the state in a single pass."""
    global _CUBE_OP
    if _CUBE_OP is not None:
        return _CUBE_OP
    import concourse.dve_ops as dve_ops

    for o in dve_ops.OPS:
        if o.name == "CUBE_MUL_ANT":
            _CUBE_OP = o
            return o
    from concourse.dve_ops import DveOp
    from concourse.dve_spec import C0, Spec, Src0, Src1, lower
    from concourse.dve_uop import DveOpSpec

    body = ((Src0 * Src0 - C0) * Src0) * Src1
    spec = Spec(
        body=body,
        reference=lambda in0, in1, c0, c1, c2: (
            ((in0.astype(np.float32) * in0 - c0) * in0) * in1
        ),
    )
    row = max(dve_ops._SUB_OPCODE_FOR_NAME.values()) + 1
    shas = {}
    for ver in ("v3", "v4"):
        uops = lower(spec, ver=ver)
        shas[ver] = DveOpSpec(
            name="CUBE_MUL_ANT", opcode=row, uops=uops, rd1_en=True
        ).sha(ver)
    op = DveOp("CUBE_MUL_ANT", spec, subdim=False, uops_sha=shas)
    dve_ops.OPS.append(op)
    dve_ops._SUB_OPCODE_FOR_NAME[op.name] = row
    dve_ops.CUSTOM_DVE_SPECS[op.name] = spec
    _CUBE_OP = op
    return op


# ---------------- device program ----------------
_PROGRAM_CACHE = {}

f16 = mybir.dt.float16
f32 = mybir.dt.float32

# patch chunks: (col0, row0, nrows)
CHUNKS = []
_r = 0
while _r < OH:
    nr = min(ROWS_PER_CHUNK, OH - _r)
    CHUNKS.append((_r * OH, _r, nr))
    _r += nr
NCH = len(CHUNKS)  # 8
NPIXCH = NPIX // PIXCH  # 4


def _build_program():
    key = "v2"
    if key in _PROGRAM_CACHE:
        return _PROGRAM_CACHE[key]

    nc = bacc.Bacc("TRN2", target_bir_lowering=False, debug=False)
    x_d = nc.dram_tensor("xin", [8, NPIX], f16, kind="ExternalInput").ap()
    wm_d = nc.dram_tensor("wm", [128, WCOLS], f16, kind="ExternalInput").ap()
    bias_d = nc.dram_tensor("bias32", [128, 2], f32, kind="ExternalInput").ap()
    z_d = nc.dram_tensor("zout", [16, P], f16, kind="ExternalOutput").ap()

    CUBE = _register_cube_mul()
    SIN = mybir.ActivationFunctionType.Sin
    COPY = mybir.ActivationFunctionType.Copy
    SQUARE = mybir.ActivationFunctionType.Square

    with tile.TileContext(nc) as tc:
        from contextlib import ExitStack

        with ExitStack() as ctx:
            const_pool = ctx.enter_context(tc.tile_pool(name="const", bufs=1))
            v_pool = ctx.enter_context(tc.tile_pool(name="v", bufs=2))
            ysb_pool = ctx.enter_context(tc.tile_pool(name="ysb", bufs=4))
            m_pool = ctx.enter_context(tc.tile_pool(name="m", bufs=4))
            sq_pool = ctx.enter_context(tc.tile_pool(name="sq", bufs=2))
            zs_pool = ctx.enter_context(tc.tile_pool(name="zs", bufs=2))
            aps_pool = ctx.enter_context(
                tc.tile_pool(name="aps", bufs=1, space="PSUM")
            )
            yps = [
                ctx.enter_context(tc.tile_pool(name=f"y{i}", bufs=1, space="PSUM"))
                for i in range(N_CHAINS)
            ]

            wm_sb = const_pool.tile([128, WCOLS], f16)
            x_sb = const_pool.tile([8, NPIX], f16)
            bias_sb = const_pool.tile([128, 2], f32)
            ones_sb = const_pool.tile([128, PIXCH], f16)
            # Phi pixel arrays: plane 0 = -cos(alpha)/4, plane 1 = -sin/4
            phi = const_pool.tile([128, 2, HW, HW], f16)
            zbuf = const_pool.tile([16, P], f16)

            def wma(c0, n=128):
                return wm_sb[:, c0 : c0 + n]

            chain = {}

            def do_init(_a, _b):
                nc.gpsimd.dma_start(bias_sb[:], bias_d[:])
                nc.vector.memset(ones_sb[:], 1.0)

            def do_wm_dma(i, _p):
                # alpha cols first on the sync queue (critical path);
                # the rest via gpsimd SWDGE in parallel
                if i == 0:
                    nc.scalar.dma_start(wm_sb[:, 0:128], wm_d[:, 0:128])
                else:
                    nc.gpsimd.dma_start(
                        wm_sb[:, 128:WCOLS], wm_d[:, 128:WCOLS]
                    )

            def do_xdma(i, _p):
                nc.sync.dma_start(
                    x_sb[:, i * PIXCH : (i + 1) * PIXCH],
                    x_d[:, i * PIXCH : (i + 1) * PIXCH],
                )

            def do_pix(i, _p):
                a_ps = aps_pool.tile([128, PIXCH], f32, tag="aps", name=f"a{i}")
                for h in range(PIXCH // 512):
                    nc.tensor.matmul(
                        a_ps[:, h * 512 : (h + 1) * 512],
                        wm_sb[0:8, A_ALPHA : A_ALPHA + 128],
                        x_sb[:, i * PIXCH + h * 512 : i * PIXCH + (h + 1) * 512],
                        start=True,
                        stop=True,
                    )
                vc = v_pool.tile([128, PIXCH], f16, tag="vc", name=f"vc{i}")
                vs = v_pool.tile([128, PIXCH], f16, tag="vs", name=f"vs{i}")
                nc.scalar.activation(vc[:], a_ps[:], SIN, bias=bias_sb[:, 0:1])
                nc.scalar.activation(vs[:], a_ps[:], SIN, bias=bias_sb[:, 1:2])
                r = i * (PIXCH // HW)
                nrr = PIXCH // HW
                nc.vector._custom_dve(
                    CUBE,
                    out=phi[:, 0, r : r + nrr, :],
                    in0=vc[:],
                    in1=ones_sb[:],
                    s0=0.75,
                )
                nc.vector._custom_dve(
                    CUBE,
                    out=phi[:, 1, r : r + nrr, :],
                    in0=vs[:],
                    in1=ones_sb[:],
                    s0=0.75,
                )

            def do_pos(ci, p):
                c0, r0, nr = CHUNKS[ci]
                C = nr * OH
                dy, dx = p // 3, p % 3
                ypool = yps[ci % N_CHAINS]
                y_new = ypool.tile(
                    [128, ROWS_PER_CHUNK, OH],
                    f32,
                    tag=f"y{ci % N_CHAINS}",
                    name=f"y{ci}_{p}",
                )
                if p == 0:
                    rhsC = phi[:, 0, r0 + dy : r0 + dy + nr, dx : dx + OH]
                    rhsS = phi[:, 1, r0 + dy : r0 + dy + nr, dx : dx + OH]
                    nc.tensor.matmul(
                        y_new[:, :nr, :].opt(), wma(A_P0C), rhsC, start=True,
                        stop=False,
                    )
                    nc.tensor.matmul(
                        y_new[:, :nr, :].opt(), wma(A_P0S), rhsS, start=False,
                        stop=True,
                    )
                else:
                    y_prev = chain[ci]
                    m = m_pool.tile(
                        [128, 2, ROWS_PER_CHUNK, OH], f16, tag="m", name=f"m{ci}_{p}"
                    )
                    in0 = phi[:, :, r0 + dy : r0 + dy + nr, dx : dx + OH]
                    if p in DVE_CV_POS:
                        # fused: DVE reads the fp32 state straight from PSUM
                        # (1x mode) - no separate downconvert op
                        in1 = (
                            y_prev[:, :nr, :]
                            .unsqueeze(1)
                            .broadcast_to([128, 2, nr, OH])
                        )
                        nc.vector.tensor_mul(m[:, :, :nr, :], in0, in1)
                    else:
                        ysb = ysb_pool.tile(
                            [128, ROWS_PER_CHUNK, OH],
                            f16,
                            tag="ysb",
                            name=f"ysb{ci}_{p}",
                        )
                        nc.scalar.activation(
                            ysb[:, :nr, :].opt(), y_prev[:, :nr, :].opt(), COPY
                        )
                        in1 = (
                            ysb[:, :nr, :]
                            .unsqueeze(1)
                            .broadcast_to([128, 2, nr, OH])
                        )
                        if p in POOL_TT_POS:
                            nc.gpsimd.tensor_mul(m[:, :, :nr, :], in0, in1)
                        else:
                            nc.vector.tensor_mul(m[:, :, :nr, :], in0, in1)
                    col = A_M + (p - 1) * 256
                    nc.tensor.matmul(
                        y_new[:, :nr, :].opt(),
                        wma(col),
                        m[:, 0, :nr, :].opt(),
                        start=True,
                        stop=False,
                    )
                    nc.tensor.matmul(
                        y_new[:, :nr, :].opt(),
                        wma(col + 128),
                        m[:, 1, :nr, :].opt(),
                        start=False,
                        stop=True,
                    )
                chain[ci] = y_new

            def do_meas(ci, _p):
                c0, r0, nr = CHUNKS[ci]
                C = nr * OH
                y9 = chain.pop(ci)
                sq = sq_pool.tile(
                    [128, ROWS_PER_CHUNK, OH], f16, tag="sq", name=f"sq{ci}"
                )
                nc.scalar.activation(
                    sq[:, :nr, :].opt(), y9[:, :nr, :].opt(), SQUARE
                )
                # zq reuses the finished chain's PSUM bank
                ypool = yps[ci % N_CHAINS]
                zq = ypool.tile(
                    [16, 512], f32, tag=f"y{ci % N_CHAINS}", name=f"zq{ci}"
                )
                nc.tensor.matmul(
                    zq[:, :C],
                    wma(A_Z, 16),
                    sq[:, :nr, :].opt(),
                    start=True,
                    stop=True,
                )
                nc.scalar.activation(zbuf[:, c0 : c0 + C], zq[:, :C], COPY)
                nc.sync.dma_start(z_d[:, c0 : c0 + C], zbuf[:, c0 : c0 + C])

            # time-ordered emission; pixel chunks lead their consumers
            OFF = 1.45
            events = [(-11.0, 0, 0, 0, do_init)]
            events.append((-10.5, 0, 0, 0, do_wm_dma))
            events.append((-10.4, 0, 1, 0, do_wm_dma))
            for j in range(NPIXCH):
                t = max(-8.0, (j - 1) * OFF - 3.0)
                events.append((t - 0.4, 0, j, 0, do_xdma))
                events.append((t, 1, j, 0, do_pix))
            for ci in range(NCH):
                for p in range(NPOS):
                    events.append((ci * OFF + p, 2, ci, p, do_pos))
                events.append((ci * OFF + NPOS, 3, ci, 0, do_meas))
            events.sort(key=lambda e: (e[0], e[1], e[2]))
            for _t, _k, ci, p, fn in events:
                fn(ci, p)

    nc.compile()
    _PROGRAM_CACHE[key] = nc
    return nc


# ---------------- entry point ----------------
_BIAS32 = np.zeros((128, 2), dtype=np.float32)
_BIAS32[:, 0] = np.pi / 6


def kernel(x, weights):
    x = np.asarray(x, dtype=np.float32)
    weights = np.asarray(weights, dtype=np.float32)
    wm = _build_wm(weights)

    nc = _build_program()
    in_maps = []
    for c in range(N_CORES):
        xc = np.ascontiguousarray(
            x[2 * c : 2 * c + 2].reshape(8, NPIX), dtype=np.float16
        )
        in_maps.append({"xin": xc, "wm": wm, "bias32": _BIAS32})
    res = run_bass_kernel_spmd(nc, in_maps, list(range(N_CORES)))

    out = np.zeros((B, 2 * IN_CH, OH, OH), dtype=np.float32)
    for c in range(N_CORES):
        z = np.asarray(res.results[c]["zout"]).astype(np.float32)  # (16, P)
        for k in range(2):
            for img in range(2):
                b = 2 * c + img
                for q in range(IN_CH):
                    out[b, k * IN_CH + q] = z[k * 8 + img * 4 + q].reshape(OH, OH)
    return out
